# revision 3
# baseline (speedup 1.0000x reference)
"""DeepSeek-MoE layer on 8 Trainium2 NeuronCores (v2: fp16 + XBAR transposes).

Sharding: expert-parallel routed experts (1 expert/core, full x replicated so
no token all-to-all is needed), tensor-parallel shared expert (I_S split 8
ways), single ReduceScatter (16 pipelined fp16 chunks) combines routed +
shared partial sums and leaves each core holding 1/16*1/8 row chunks.

v2 changes vs v1 (4.29ms -> target ~1.4ms):
  - all FFN math in fp16 (weights + activations, fp32 PSUM accum); numpy sim
    gives rel_l2 5.5e-4 vs the fp32 reference. fp16 also halves PE power vs
    f32r LOW_HIGH mode, relieving the 50%/81% HW throttle seen in the trace.
  - x transposed by the DMA XBAR (dma_start_transpose) straight from DRAM,
    killing all PE transposes of x and ~300us of PSUM->SBUF vector copies.
  - router logits need fp32-grade precision for stable top-2: z =
    xh*wh + xl*wh + xh*wl with (hi, lo) fp16 splits (error ~1e-5, no flips).
  - no xT DRAM round-trip: P3 re-transposes x from DRAM per supertile.
  - shared-expert supertiles 0,1 computed during the R3 dispatch latency.
  - P3 weights prefetched at kernel start; eout/chunks/RS in fp16; RS output
    in Shared scratchpad; final fp16->f32 cast by SWDGE DMA.
"""

import numpy as np
import ml_dtypes

import concourse.bass as bass
import concourse.mybir as mybir
import concourse.tile as tile
from concourse import bacc
from concourse.masks import make_identity
from concourse.bass_utils import run_bass_kernel_spmd

dt = mybir.dt
Alu = mybir.AluOpType
Act = mybir.ActivationFunctionType

P = 128

FULL_CFG = dict(T=8192, H=1024, E=8, IR=4096, IS=8192, CAP=1280, NC=8, NCH=16)


def build(cfg):
    T, H, E, IR, IS, CAP, NCORES = (
        cfg[k] for k in ("T", "H", "E", "IR", "IS", "CAP", "NC")
    )
    J = T // P            # 128-token tiles
    HC = H // P           # h chunks of 128
    ISH = IS // NCORES    # shared-expert intermediate shard
    CT = CAP // P         # capacity tiles of 128 slots
    NCH = cfg.get("NCH", 16)  # reduce-scatter chunks
    JCH = J // NCH        # token tiles per RS chunk
    ST = 4                # token tiles per shared-expert supertile
    STT = ST * P
    NST = J // ST
    assert J % NCH == 0 and J % ST == 0 and CAP % P == 0
    IRT = IR // P
    ISHT = ISH // P
    NHT = H // 512
    TC = T // STT         # router chunks of STT tokens

    nc = bacc.Bacc(None)

    f32, f16, i32 = dt.float32, dt.float16, dt.int32

    xh_in = nc.declare_dram_parameter("xh", [T, H], f16, isOutput=False)
    xl_in = nc.declare_dram_parameter("xl", [T, H], f16, isOutput=False)
    rwh_in = nc.declare_dram_parameter("rwh", [H, E], f16, isOutput=False)
    rwl_in = nc.declare_dram_parameter("rwl", [H, E], f16, isOutput=False)
    rg_in = nc.declare_dram_parameter("rg", [H, IR], f16, isOutput=False)
    ru_in = nc.declare_dram_parameter("ru", [H, IR], f16, isOutput=False)
    rd_in = nc.declare_dram_parameter("rd", [IR, H], f16, isOutput=False)
    sg_in = nc.declare_dram_parameter("sg", [H, ISH], f16, isOutput=False)
    su_in = nc.declare_dram_parameter("su", [H, ISH], f16, isOutput=False)
    sd_in = nc.declare_dram_parameter("sd", [ISH, H], f16, isOutput=False)
    eidf_in = nc.declare_dram_parameter("eidf", [P, J], f32, isOutput=False)
    iotaf_in = nc.declare_dram_parameter("iotaf", [P, J * E], f32, isOutput=False)
    tokidf_in = nc.declare_dram_parameter("tokidf", [P, J], f32, isOutput=False)
    slotiota_in = nc.declare_dram_parameter("slotiota", [P, CAP // P], f32, isOutput=False)
    utri_in = nc.declare_dram_parameter("utri", [J, J], f32, isOutput=False)
    o_out = nc.declare_dram_parameter("o", [NCH, T // NCH // NCORES, H], f32,
                                      isOutput=True)

    replica_groups = [list(range(NCORES))]

    with tile.TileContext(nc) as tc:
        with tc.tile_pool(name="dram", bufs=1, space="DRAM") as drp, \
             tc.tile_pool(name="pers", bufs=1) as pers:
            vlin_dram = drp.tile([T], f32)
            dlin_dram = drp.tile([CAP], f32)
            eout_dram = drp.tile([CAP, H], f16)
            chunk_dram = [drp.tile([T // NCH, H], f16, name=f"partial{k}")
                          for k in range(NCH)]
            rsout_dram = [drp.tile([T // NCH // NCORES, H], f16,
                                   name=f"rsout{k}") for k in range(NCH)]

            ident = pers.tile([P, P], f32)
            make_identity(nc, ident[:])
            rwh_sb = pers.tile([P, HC, E], f16)
            nc.sync.dma_start(out=rwh_sb[:],
                              in_=rwh_in[:].rearrange("(hc hp) e -> hp hc e", hp=P))
            rwl_sb = pers.tile([P, HC, E], f16)
            nc.sync.dma_start(out=rwl_sb[:],
                              in_=rwl_in[:].rearrange("(hc hp) e -> hp hc e", hp=P))
            # prefetch shared-expert gate/up weights (used from R3-prefill on)
            sgw = pers.tile([P, HC, ISH], f16)
            nc.sync.dma_start(
                out=sgw[:], in_=sg_in[:].rearrange("(hc hp) i -> hp hc i", hp=P))
            suw = pers.tile([P, HC, ISH], f16)
            nc.sync.dma_start(
                out=suw[:], in_=su_in[:].rearrange("(hc hp) i -> hp hc i", hp=P))
            eidf_sb = pers.tile([P, J], f32)
            nc.sync.dma_start(out=eidf_sb[:], in_=eidf_in[:])
            iotaf_sb = pers.tile([P, J, E], f32)
            nc.sync.dma_start(out=iotaf_sb[:],
                              in_=iotaf_in[:].rearrange("p (j e) -> p j e", e=E))
            tokidf_sb = pers.tile([P, J], f32)
            nc.sync.dma_start(out=tokidf_sb[:], in_=tokidf_in[:])
            slotiota_sb = pers.tile([P, CT], f32)
            nc.sync.dma_start(out=slotiota_sb[:], in_=slotiota_in[:])
            utri_sb = pers.tile([J, J], f32)
            nc.sync.dma_start(out=utri_sb[:], in_=utri_in[:])

            eps_sb = pers.tile([P, 1], f32)
            nc.vector.memset(eps_sb[:], float(np.finfo(np.float32).eps))

            z_all = pers.tile([P, J, E], f32)
            ss_all = pers.tile([P, J], f32)
            rms_all = pers.tile([P, J], f32)
            wv_pm = pers.tile([P, J], f32)       # combine weight per token
            slotg_i32 = pers.tile([P, J], i32)   # clamped slot for gather
            disp_sb = pers.tile([P, CT], i32)    # dispatch token ids
            hsT01 = [pers.tile([P, ISHT, STT], f16, name=f"hsT0{s}")
                     for s in range(2)]          # prefilled shared supertiles

            def shared_gu(st, xTs, hsT, spool, pspool):
                """gate/up + silu for one shared-expert supertile."""
                for it in range(ISHT):
                    ps_g3 = pspool.tile([P, STT], f32, tag="ps_g3", bufs=2)
                    ps_u3 = pspool.tile([P, STT], f32, tag="ps_u3", bufs=2)
                    for hc in range(HC):
                        nc.tensor.matmul(
                            out=ps_g3[:], lhsT=sgw[:, hc, it * P:(it + 1) * P],
                            rhs=xTs[:, hc, :],
                            start=(hc == 0), stop=(hc == HC - 1))
                        nc.tensor.matmul(
                            out=ps_u3[:], lhsT=suw[:, hc, it * P:(it + 1) * P],
                            rhs=xTs[:, hc, :],
                            start=(hc == 0), stop=(hc == HC - 1))
                    sil3 = spool.tile([P, STT], f32, tag="sil3")
                    nc.scalar.activation(out=sil3[:], in_=ps_g3[:],
                                         func=Act.Sigmoid)
                    nc.vector.tensor_tensor(out=sil3[:], in0=sil3[:],
                                            in1=ps_g3[:], op=Alu.mult)
                    nc.vector.tensor_tensor(out=hsT[:, it, :], in0=sil3[:],
                                            in1=ps_u3[:], op=Alu.mult)

            # ---------------- P1: XBAR transpose + fp16-split router --------
            with tc.tile_pool(name="p1", bufs=2) as p1, \
                 tc.tile_pool(name="p1ps", bufs=1, space="PSUM") as p1ps:
                for tci in range(TC):
                    t0 = tci * STT
                    xTh = p1.tile([P, HC, STT], f16, tag="xTh")
                    nc.sync.dma_start_transpose(out=xTh[:],
                                                in_=xh_in[t0:t0 + STT, :])
                    xTl = p1.tile([P, HC, STT], f16, tag="xTl")
                    nc.sync.dma_start_transpose(out=xTl[:],
                                                in_=xl_in[t0:t0 + STT, :])
                    zps = p1ps.tile([E, STT], f32, tag="zps", bufs=2)
                    pairs = [(rwh_sb, xTh), (rwh_sb, xTl), (rwl_sb, xTh)]
                    n_mm = len(pairs) * HC
                    k = 0
                    for (wsb, xsb) in pairs:
                        for hc in range(HC):
                            nc.tensor.matmul(out=zps[:], lhsT=wsb[:, hc, :],
                                             rhs=xsb[:, hc, :],
                                             start=(k == 0), stop=(k == n_mm - 1))
                            k += 1
                    ztmp = p1.tile([E, STT], f32, tag="ztmp")
                    nc.vector.tensor_copy(out=ztmp[:], in_=zps[:])
                    for q in range(ST):
                        ztr = p1ps.tile([P, E], f32, tag="ztr", bufs=2)
                        nc.tensor.transpose(ztr[:], ztmp[:, q * P:(q + 1) * P],
                                            ident[:E, :E])
                        nc.vector.tensor_copy(out=z_all[:, tci * ST + q, :],
                                              in_=ztr[:])
                    # sum(x^2) stream for the RMS scale (scalar engine)
                    for q in range(ST):
                        j = tci * ST + q
                        xr = p1.tile([P, H], f16, tag="xr", bufs=3)
                        nc.sync.dma_start(out=xr[:],
                                          in_=xh_in[j * P:(j + 1) * P, :])
                        sq = p1.tile([P, H], f32, tag="sq")
                        nc.scalar.activation(out=sq[:], in_=xr[:],
                                             func=Act.Square,
                                             accum_out=ss_all[:, j:j + 1])

            # ---------------- R2: top-2 + weights ---------------------------
            with tc.tile_pool(name="r2", bufs=1) as r2, \
                 tc.tile_pool(name="r2ps", bufs=1, space="PSUM") as r2ps:
                srt = r2.tile([P, J], f32)
                nc.scalar.activation(out=srt[:], in_=ss_all[:], func=Act.Sqrt,
                                     scale=1.0 / H, bias=eps_sb[:])
                nc.vector.reciprocal(out=rms_all[:], in_=srt[:])

                m1 = r2.tile([P, J], f32)
                m2 = r2.tile([P, J], f32)
                idx1 = r2.tile([P, J], f32)
                idx2 = r2.tile([P, J], f32)
                eq = r2.tile([P, J, E], f32)
                tmpje = r2.tile([P, J, E], f32)
                tmp = r2.tile([P, J], f32)
                rw1 = r2.tile([P, J], f32)
                rw2 = r2.tile([P, J], f32)

                nc.vector.tensor_reduce(out=m1[:], in_=z_all[:],
                                        axis=mybir.AxisListType.X, op=Alu.max)
                m1b = m1[:].rearrange("p j -> p j ()").to_broadcast([P, J, E])
                nc.vector.tensor_tensor(out=eq[:], in0=z_all[:], in1=m1b,
                                        op=Alu.is_ge)
                # idx1 = min over e of (eq ? iota : 9)
                nc.vector.scalar_tensor_tensor(out=tmpje[:], in0=iotaf_sb[:],
                                               scalar=-9.0, in1=eq[:],
                                               op0=Alu.add, op1=Alu.mult)
                nc.vector.tensor_scalar_add(tmpje[:], tmpje[:], 9.0)
                nc.vector.tensor_reduce(out=idx1[:], in_=tmpje[:],
                                        axis=mybir.AxisListType.X, op=Alu.min)
                # mask out the top-1 positions, then find second max
                nc.vector.scalar_tensor_tensor(out=eq[:], in0=eq[:],
                                               scalar=-1e30, in1=z_all[:],
                                               op0=Alu.mult, op1=Alu.add)
                nc.vector.tensor_reduce(out=m2[:], in_=eq[:],
                                        axis=mybir.AxisListType.X, op=Alu.max)
                m2b = m2[:].rearrange("p j -> p j ()").to_broadcast([P, J, E])
                nc.vector.tensor_tensor(out=eq[:], in0=eq[:], in1=m2b,
                                        op=Alu.is_ge)
                nc.vector.scalar_tensor_tensor(out=tmpje[:], in0=iotaf_sb[:],
                                               scalar=-9.0, in1=eq[:],
                                               op0=Alu.add, op1=Alu.mult)
                nc.vector.tensor_scalar_add(tmpje[:], tmpje[:], 9.0)
                nc.vector.tensor_reduce(out=idx2[:], in_=tmpje[:],
                                        axis=mybir.AxisListType.X, op=Alu.min)

                # rw1 = sigmoid((m1-m2)*rms), rw2 = 1-rw1
                nc.vector.tensor_sub(tmp[:], m1[:], m2[:])
                nc.vector.tensor_mul(tmp[:], tmp[:], rms_all[:])
                nc.scalar.activation(out=rw1[:], in_=tmp[:], func=Act.Sigmoid)
                nc.vector.tensor_scalar(rw2[:], rw1[:], -1.0, 1.0,
                                        op0=Alu.mult, op1=Alu.add)

                se1 = r2.tile([P, J], f32)
                se2 = r2.tile([P, J], f32)
                sel = r2.tile([P, J], f32)
                nc.vector.tensor_tensor(out=se1[:], in0=idx1[:], in1=eidf_sb[:],
                                        op=Alu.is_equal)
                nc.vector.tensor_tensor(out=se2[:], in0=idx2[:], in1=eidf_sb[:],
                                        op=Alu.is_equal)
                nc.vector.tensor_add(sel[:], se1[:], se2[:])
                nc.vector.tensor_mul(se1[:], se1[:], rw1[:])
                nc.vector.tensor_mul(se2[:], se2[:], rw2[:])
                nc.vector.tensor_add(wv_pm[:], se1[:], se2[:])

                # ------------- R3: capacity dispatch ------------------------
                selT_ps = r2ps.tile([J, P], f32)
                nc.tensor.transpose(selT_ps[:], sel[:], ident[:])
                selT = r2.tile([J, P], f32)
                nc.vector.tensor_copy(out=selT[:], in_=selT_ps[:])
                zerosT = r2.tile([J, P], f32)
                nc.vector.memset(zerosT[:], 0.0)
                rowsum = r2.tile([J, 1], f32)
                nc.vector.tensor_reduce(out=rowsum[:], in_=selT[:],
                                        axis=mybir.AxisListType.X, op=Alu.add)
                offs_ps = r2ps.tile([J, 1], f32)
                nc.tensor.matmul(out=offs_ps[:], lhsT=utri_sb[:], rhs=rowsum[:],
                                 start=True, stop=True)
                scanT = r2.tile([J, P], f32)
                nc.vector.tensor_tensor_scan(out=scanT[:], data0=selT[:],
                                             data1=zerosT[:],
                                             initial=offs_ps[:],
                                             op0=Alu.add, op1=Alu.add)
                nc.vector.tensor_scalar_add(scanT[:], scanT[:], -1.0)
                slot_ps = r2ps.tile([P, J], f32)
                nc.tensor.transpose(slot_ps[:], scanT[:], ident[:J, :J])
                slot_pm = r2.tile([P, J], f32)
                nc.vector.tensor_copy(out=slot_pm[:], in_=slot_ps[:])

                # wv *= (slot < CAP)
                gate = r2.tile([P, J], f32)
                nc.vector.tensor_scalar(gate[:], slot_pm[:], float(CAP), None,
                                        op0=Alu.is_lt)
                nc.vector.tensor_mul(wv_pm[:], wv_pm[:], gate[:])
                # gather slot: clamp to [0, CAP-1]
                sg_f = r2.tile([P, J], f32)
                nc.vector.tensor_scalar(sg_f[:], slot_pm[:], 0.0, float(CAP - 1),
                                        op0=Alu.max, op1=Alu.min)
                nc.vector.tensor_copy(out=slotg_i32[:], in_=sg_f[:])
                # dispatch build: stream-compact (sel & slot<CAP ? tokid : -1)
                # in token order via gpsimd sparse_gather. The capacity cap
                # keeps the found count <= CAP so the ucode cannot overrun
                # its [16, CAP/16] output.
                selcap = r2.tile([P, J], f32)
                nc.vector.tensor_mul(selcap[:], sel[:], gate[:])
                val_pm = r2.tile([P, J], f32)
                nc.vector.scalar_tensor_tensor(out=val_pm[:], in0=tokidf_sb[:],
                                               scalar=1.0, in1=selcap[:],
                                               op0=Alu.add, op1=Alu.mult)
                nc.vector.tensor_scalar_add(val_pm[:], val_pm[:], -1.0)
                nc.sync.dma_start(
                    out=vlin_dram[:].rearrange("(j p) -> p j", p=P),
                    in_=val_pm[:])
                v16 = r2.tile([16, T // 16], f32)
                nc.sync.dma_start(out=v16[:],
                                  in_=vlin_dram[:].rearrange("(f p) -> p f", p=16))
                d16 = r2.tile([16, CAP // 16], f32)
                nfound = r2.tile([1, 1], dt.uint32)
                nc.gpsimd.sparse_gather(out=d16[:], in_=v16[:],
                                        num_found=nfound[:])
                nc.sync.dma_start(
                    out=dlin_dram[:].rearrange("(f p) -> p f", p=16),
                    in_=d16[:])
                dispf = r2.tile([P, CT], f32)
                nc.sync.dma_start(
                    out=dispf[:],
                    in_=dlin_dram[:].rearrange("(ct p) -> p ct", p=P))
                nfoundf = r2.tile([1, 1], f32)
                nc.vector.tensor_copy(out=nfoundf[:], in_=nfound[:])
                cntb = r2.tile([P, 1], f32)
                nc.gpsimd.partition_broadcast(cntb[:], nfoundf[:])
                cgate = r2.tile([P, CT], dt.uint8)
                nc.vector.tensor_scalar(cgate[:], slotiota_sb[:], cntb[:],
                                        None, op0=Alu.is_lt)
                zct = r2.tile([P, CT], f32)
                nc.vector.memset(zct[:], 0.0)
                dsafe = r2.tile([P, CT], f32)
                nc.vector.select(out=dsafe[:], mask=cgate[:], on_true=dispf[:],
                                 on_false=zct[:])
                nc.vector.tensor_copy(out=disp_sb[:], in_=dsafe[:])

            # ------------- prefill shared supertiles 0,1 (hides R3) ---------
            with tc.tile_pool(name="pf", bufs=2) as pf, \
                 tc.tile_pool(name="pfps", bufs=1, space="PSUM") as pfps:
                for st in range(2):
                    xTs = pf.tile([P, HC, STT], f16, tag="xTs")
                    nc.sync.dma_start_transpose(
                        out=xTs[:], in_=xh_in[st * STT:(st + 1) * STT, :])
                    shared_gu(st, xTs, hsT01[st], pf, pfps)

            # ---------------- P2: routed expert FFN -------------------------
            with tc.tile_pool(name="p2", bufs=3) as p2, \
                 tc.tile_pool(name="p2big", bufs=1) as p2big:
                xgT = p2big.tile([P, HC, CAP], f16)
                hT = p2big.tile([P, IRT, CAP], f16)
                with tc.tile_pool(name="p2psA", bufs=1, space="PSUM") as psa:
                    for ct in range(CT):
                        xg_sb = p2.tile([P, H], f16, tag="xg_sb")
                        nc.gpsimd.indirect_dma_start(
                            out=xg_sb[:], out_offset=None,
                            in_=xh_in[:, :],
                            in_offset=bass.IndirectOffsetOnAxis(
                                ap=disp_sb[:, ct:ct + 1], axis=0))
                        nc.sync.dma_start_transpose(
                            out=xgT[:, :, ct * P:(ct + 1) * P], in_=xg_sb[:])

                    cchunks = []
                    c0 = 0
                    while c0 < CAP:
                        cw = min(512, CAP - c0)
                        cchunks.append((c0, cw))
                        c0 += cw
                    for irt in range(IRT):
                        rgw = p2.tile([P, HC, P], f16, tag="rgw")
                        nc.sync.dma_start(
                            out=rgw[:],
                            in_=rg_in[:, irt * P:(irt + 1) * P].rearrange(
                                "(hc hp) i -> hp hc i", hp=P))
                        ruw = p2.tile([P, HC, P], f16, tag="ruw")
                        nc.sync.dma_start(
                            out=ruw[:],
                            in_=ru_in[:, irt * P:(irt + 1) * P].rearrange(
                                "(hc hp) i -> hp hc i", hp=P))
                        for (c0, cw) in cchunks:
                            ps_g = psa.tile([P, 512], f32, tag="ps_g", bufs=2)
                            ps_u = psa.tile([P, 512], f32, tag="ps_u", bufs=2)
                            for hc in range(HC):
                                nc.tensor.matmul(
                                    out=ps_g[:, :cw], lhsT=rgw[:, hc, :],
                                    rhs=xgT[:, hc, c0:c0 + cw],
                                    start=(hc == 0), stop=(hc == HC - 1))
                                nc.tensor.matmul(
                                    out=ps_u[:, :cw], lhsT=ruw[:, hc, :],
                                    rhs=xgT[:, hc, c0:c0 + cw],
                                    start=(hc == 0), stop=(hc == HC - 1))
                            sil = p2.tile([P, 512], f32, tag="sil")
                            nc.scalar.activation(out=sil[:, :cw],
                                                 in_=ps_g[:, :cw],
                                                 func=Act.Sigmoid)
                            nc.vector.tensor_tensor(
                                out=sil[:, :cw], in0=sil[:, :cw],
                                in1=ps_g[:, :cw], op=Alu.mult)
                            nc.vector.tensor_tensor(
                                out=hT[:, irt, c0:c0 + cw], in0=sil[:, :cw],
                                in1=ps_u[:, :cw], op=Alu.mult)

                # down: eout[c, h] = sum_ir h[c, ir] * rd[ir, h]
                with tc.tile_pool(name="p2psD", bufs=1, space="PSUM") as psd:
                    ct0 = 0
                    while ct0 < CT:
                        blk = min(4, CT - ct0)
                        ps_d = [[psd.tile([P, 512], f32, tag=f"d{i}{hn}",
                                          name=f"d{i}{hn}")
                                 for hn in range(NHT)] for i in range(blk)]
                        for ic in range(IRT):
                            rdw = p2.tile([P, H], f16, tag="rdw")
                            nc.sync.dma_start(
                                out=rdw[:], in_=rd_in[ic * P:(ic + 1) * P, :])
                            for i in range(blk):
                                for hn in range(NHT):
                                    nc.tensor.matmul(
                                        out=ps_d[i][hn][:],
                                        lhsT=hT[:, ic,
                                                (ct0 + i) * P:(ct0 + i + 1) * P],
                                        rhs=rdw[:, hn * 512:(hn + 1) * 512],
                                        start=(ic == 0), stop=(ic == IRT - 1))
                        for i in range(blk):
                            eo = p2.tile([P, H], f16, tag="eo")
                            for hn in range(NHT):
                                nc.vector.tensor_copy(
                                    out=eo[:, hn * 512:(hn + 1) * 512],
                                    in_=ps_d[i][hn][:])
                            nc.sync.dma_start(
                                out=eout_dram[(ct0 + i) * P:(ct0 + i + 1) * P, :],
                                in_=eo[:])
                        ct0 += blk

            # ---------------- P3: shared expert + combine + RS --------------
            with tc.tile_pool(name="p3w", bufs=1) as p3w, \
                 tc.tile_pool(name="p3", bufs=2) as p3, \
                 tc.tile_pool(name="p3ps", bufs=1, space="PSUM") as p3ps:
                sdw = p3w.tile([P, ISHT, H], f16)
                nc.sync.dma_start(
                    out=sdw[:],
                    in_=sd_in[:].rearrange("(it ip) h -> ip it h", ip=P))

                for st in range(NST):
                    if st < 2:
                        hsT = hsT01[st]
                    else:
                        xTs = p3.tile([P, HC, STT], f16, tag="xTs")
                        nc.sync.dma_start_transpose(
                            out=xTs[:], in_=xh_in[st * STT:(st + 1) * STT, :])
                        hsT = p3.tile([P, ISHT, STT], f16, tag="hsT")
                        shared_gu(st, xTs, hsT, p3, p3ps)
                    for ts in range(ST):
                        j = st * ST + ts
                        gath = p3.tile([P, H], f16, tag="gath")
                        nc.gpsimd.indirect_dma_start(
                            out=gath[:], out_offset=None,
                            in_=eout_dram[:, :],
                            in_offset=bass.IndirectOffsetOnAxis(
                                ap=slotg_i32[:, j:j + 1], axis=0))
                        outt = p3.tile([P, H], f16, tag="outt")
                        for hn in range(NHT):
                            ps_d3 = p3ps.tile([P, 512], f32, tag="ps_d3",
                                              bufs=2)
                            for it in range(ISHT):
                                nc.tensor.matmul(
                                    out=ps_d3[:],
                                    lhsT=hsT[:, it, ts * P:(ts + 1) * P],
                                    rhs=sdw[:, it, hn * 512:(hn + 1) * 512],
                                    start=(it == 0), stop=(it == ISHT - 1))
                            nc.vector.scalar_tensor_tensor(
                                out=outt[:, hn * 512:(hn + 1) * 512],
                                in0=gath[:, hn * 512:(hn + 1) * 512],
                                scalar=wv_pm[:, j:j + 1],
                                in1=ps_d3[:],
                                op0=Alu.mult, op1=Alu.add)
                        k = j // JCH
                        r = j % JCH
                        nc.sync.dma_start(
                            out=chunk_dram[k][r * P:(r + 1) * P, :],
                            in_=outt[:])
                    if (st + 1) % (NST // NCH) == 0:
                        k = (st + 1) // (NST // NCH) - 1
                        nc.gpsimd.collective_compute(
                            "ReduceScatter", Alu.add,
                            replica_groups=replica_groups,
                            ins=[chunk_dram[k][:]],
                            outs=[rsout_dram[k][:]])
                        # fp16 -> f32 cast on the way out (SWDGE)
                        nc.gpsimd.dma_start(out=o_out[k], in_=rsout_dram[k][:])

    nc.finalize()
    return nc


_NC_CACHE = {}


def _get_nc(key="full"):
    if key not in _NC_CACHE:
        _NC_CACHE[key] = build(FULL_CFG)
    return _NC_CACHE[key]


def make_in_maps(inputs, cfg=FULL_CFG):
    T, H, E, IR, IS, CAP, NCORES = (
        cfg[k] for k in ("T", "H", "E", "IR", "IS", "CAP", "NC"))
    J = T // P
    ISH = IS // NCORES
    f16 = np.float16
    x = np.ascontiguousarray(np.asarray(inputs["x"], np.float32).reshape(T, H))
    xh = x.astype(f16)
    xl = (x - xh.astype(np.float32)).astype(f16)
    rw = np.asarray(inputs["router_w"], np.float32)
    rwt = np.ascontiguousarray(rw.T)
    rwh = rwt.astype(f16)
    rwl = (rwt - rwh.astype(np.float32)).astype(f16)
    rg = np.asarray(inputs["rg"], np.float32)
    ru = np.asarray(inputs["ru"], np.float32)
    rd = np.asarray(inputs["rd"], np.float32)
    sg = np.asarray(inputs["sg"], np.float32)
    su = np.asarray(inputs["su"], np.float32)
    sd = np.asarray(inputs["sd"], np.float32)

    tokidf = np.ascontiguousarray(
        np.arange(T, dtype=np.float32).reshape(J, P).T)      # [p, j] = 128j+p
    slotiota = np.ascontiguousarray(
        np.arange(CAP, dtype=np.float32).reshape(CAP // P, P).T)  # [p, ct]
    utri = np.triu(np.ones((J, J), np.float32), k=1)
    iotaf = np.tile(np.arange(E, dtype=np.float32), (P, J))  # [P, J*E]

    in_maps = []
    for i in range(NCORES):
        in_maps.append(dict(
            xh=xh, xl=xl, rwh=rwh, rwl=rwl,
            rg=np.ascontiguousarray(rg[i]).astype(f16),
            ru=np.ascontiguousarray(ru[i]).astype(f16),
            rd=np.ascontiguousarray(rd[i]).astype(f16),
            sg=np.ascontiguousarray(sg[:, i * ISH:(i + 1) * ISH]).astype(f16),
            su=np.ascontiguousarray(su[:, i * ISH:(i + 1) * ISH]).astype(f16),
            sd=np.ascontiguousarray(sd[i * ISH:(i + 1) * ISH, :]).astype(f16),
            eidf=np.full((P, J), float(i), np.float32),
            iotaf=iotaf, tokidf=tokidf, slotiota=slotiota, utri=utri,
        ))
    return in_maps


def assemble_output(results, cfg=FULL_CFG):
    T, H, NCORES = cfg["T"], cfg["H"], cfg["NC"]
    NCH = cfg.get("NCH", 16)
    rows_per = T // NCH // NCORES
    out = np.empty((T, H), np.float32)
    for i in range(NCORES):
        o = results[i]["o"]
        for k in range(NCH):
            base = (T // NCH) * k + rows_per * i
            out[base:base + rows_per] = o[k]
    return out


def kernel(**inputs):
    nc = _get_nc()
    in_maps = make_in_maps(inputs)
    core_ids = list(range(FULL_CFG["NC"]))
    last_err = None
    for _attempt in range(2):
        try:
            res = run_bass_kernel_spmd(nc, in_maps, core_ids, trace=False)
            break
        except Exception as e:  # transient device wedges: retry once
            last_err = e
    else:
        raise last_err
    out = assemble_output(res.results)
    B, S, H = 4, 2048, 1024
    return out.reshape(B, S, H)


# revision 4
# speedup vs baseline: 1.0792x; 1.0792x over previous
"""DeepSeek-MoE layer on 8 Trainium2 NeuronCores (v2: fp16 + XBAR transposes).

Sharding: expert-parallel routed experts (1 expert/core, full x replicated so
no token all-to-all is needed), tensor-parallel shared expert (I_S split 8
ways), single ReduceScatter (16 pipelined fp16 chunks) combines routed +
shared partial sums and leaves each core holding 1/16*1/8 row chunks.

v2 changes vs v1 (4.29ms -> target ~1.4ms):
  - all FFN math in fp16 (weights + activations, fp32 PSUM accum); numpy sim
    gives rel_l2 5.5e-4 vs the fp32 reference. fp16 also halves PE power vs
    f32r LOW_HIGH mode, relieving the 50%/81% HW throttle seen in the trace.
  - x transposed by the DMA XBAR (dma_start_transpose) straight from DRAM,
    killing all PE transposes of x and ~300us of PSUM->SBUF vector copies.
  - router logits need fp32-grade precision for stable top-2: z =
    xh*wh + xl*wh + xh*wl with (hi, lo) fp16 splits (error ~1e-5, no flips).
  - no xT DRAM round-trip: P3 re-transposes x from DRAM per supertile.
  - shared-expert supertiles 0,1 computed during the R3 dispatch latency.
  - P3 weights prefetched at kernel start; eout/chunks/RS in fp16; RS output
    in Shared scratchpad; final fp16->f32 cast by SWDGE DMA.
"""

import numpy as np
import ml_dtypes

import concourse.bass as bass
import concourse.mybir as mybir
import concourse.tile as tile
from concourse import bacc
from concourse.masks import make_identity
from concourse.bass_utils import run_bass_kernel_spmd

dt = mybir.dt
Alu = mybir.AluOpType
Act = mybir.ActivationFunctionType

P = 128

FULL_CFG = dict(T=8192, H=1024, E=8, IR=4096, IS=8192, CAP=1280, NC=8, NCH=16)


def build(cfg):
    T, H, E, IR, IS, CAP, NCORES = (
        cfg[k] for k in ("T", "H", "E", "IR", "IS", "CAP", "NC")
    )
    J = T // P            # 128-token tiles
    HC = H // P           # h chunks of 128
    ISH = IS // NCORES    # shared-expert intermediate shard
    CT = CAP // P         # capacity tiles of 128 slots
    NCH = cfg.get("NCH", 16)  # reduce-scatter chunks
    JCH = J // NCH        # token tiles per RS chunk
    ST = 4                # token tiles per shared-expert supertile
    STT = ST * P
    NST = J // ST
    assert J % NCH == 0 and J % ST == 0 and CAP % P == 0
    IRT = IR // P
    ISHT = ISH // P
    NHT = H // 512
    TC = T // STT         # router chunks of STT tokens

    nc = bacc.Bacc(None)

    f32, f16, i32 = dt.float32, dt.float16, dt.int32

    xh_in = nc.declare_dram_parameter("xh", [T, H], f16, isOutput=False)
    xht_in = nc.declare_dram_parameter("xht", [H, T], f16, isOutput=False)
    xlt_in = nc.declare_dram_parameter("xlt", [H, T], f16, isOutput=False)
    rwh_in = nc.declare_dram_parameter("rwh", [H, E], f16, isOutput=False)
    rwl_in = nc.declare_dram_parameter("rwl", [H, E], f16, isOutput=False)
    rg_in = nc.declare_dram_parameter("rg", [H, IR], f16, isOutput=False)
    ru_in = nc.declare_dram_parameter("ru", [H, IR], f16, isOutput=False)
    rd_in = nc.declare_dram_parameter("rd", [IR, H], f16, isOutput=False)
    sg_in = nc.declare_dram_parameter("sg", [H, ISH], f16, isOutput=False)
    su_in = nc.declare_dram_parameter("su", [H, ISH], f16, isOutput=False)
    sd_in = nc.declare_dram_parameter("sd", [ISH, H], f16, isOutput=False)
    eidf_in = nc.declare_dram_parameter("eidf", [P, J], f32, isOutput=False)
    iotaf_in = nc.declare_dram_parameter("iotaf", [P, J * E], f32, isOutput=False)
    tokidf_in = nc.declare_dram_parameter("tokidf", [P, J], f32, isOutput=False)
    slotiota_in = nc.declare_dram_parameter("slotiota", [P, CAP // P], f32, isOutput=False)
    utri_in = nc.declare_dram_parameter("utri", [J, J], f32, isOutput=False)
    o_out = nc.declare_dram_parameter("o", [NCH, T // NCH // NCORES, H], f32,
                                      isOutput=True)

    replica_groups = [list(range(NCORES))]

    with tile.TileContext(nc) as tc:
        with tc.tile_pool(name="dram", bufs=1, space="DRAM") as drp, \
             tc.tile_pool(name="pers", bufs=1) as pers:
            vlin_dram = drp.tile([T], f32)
            dlin_dram = drp.tile([CAP], f32)
            eout_dram = drp.tile([CAP, H], f16)
            chunk_dram = [drp.tile([T // NCH, H], f16, name=f"partial{k}")
                          for k in range(NCH)]
            rsout_dram = [drp.tile([T // NCH // NCORES, H], f16,
                                   name=f"rsout{k}") for k in range(NCH)]

            ident = pers.tile([P, P], f32)
            make_identity(nc, ident[:])
            rwh_sb = pers.tile([P, HC, E], f16)
            nc.sync.dma_start(out=rwh_sb[:],
                              in_=rwh_in[:].rearrange("(hc hp) e -> hp hc e", hp=P))
            rwl_sb = pers.tile([P, HC, E], f16)
            nc.sync.dma_start(out=rwl_sb[:],
                              in_=rwl_in[:].rearrange("(hc hp) e -> hp hc e", hp=P))
            # prefetch shared-expert gate/up weights (used from R3-prefill on)
            sgw = pers.tile([P, HC, ISH], f16)
            nc.sync.dma_start(
                out=sgw[:], in_=sg_in[:].rearrange("(hc hp) i -> hp hc i", hp=P))
            suw = pers.tile([P, HC, ISH], f16)
            nc.sync.dma_start(
                out=suw[:], in_=su_in[:].rearrange("(hc hp) i -> hp hc i", hp=P))
            eidf_sb = pers.tile([P, J], f32)
            nc.sync.dma_start(out=eidf_sb[:], in_=eidf_in[:])
            iotaf_sb = pers.tile([P, J, E], f32)
            nc.sync.dma_start(out=iotaf_sb[:],
                              in_=iotaf_in[:].rearrange("p (j e) -> p j e", e=E))
            tokidf_sb = pers.tile([P, J], f32)
            nc.sync.dma_start(out=tokidf_sb[:], in_=tokidf_in[:])
            slotiota_sb = pers.tile([P, CT], f32)
            nc.sync.dma_start(out=slotiota_sb[:], in_=slotiota_in[:])
            utri_sb = pers.tile([J, J], f32)
            nc.sync.dma_start(out=utri_sb[:], in_=utri_in[:])

            eps_sb = pers.tile([P, 1], f32)
            nc.vector.memset(eps_sb[:], float(np.finfo(np.float32).eps))

            z_all = pers.tile([P, J, E], f32)
            ss_all = pers.tile([P, J], f32)
            rms_all = pers.tile([P, J], f32)
            wv_pm = pers.tile([P, J], f32)       # combine weight per token
            slotg_i32 = pers.tile([P, J], i32)   # clamped slot for gather
            disp_sb = pers.tile([P, CT], i32)    # dispatch token ids
            NPF = 3
            hsT01 = [pers.tile([P, ISHT, STT], f16, name=f"hsT0{s}")
                     for s in range(NPF)]        # prefilled shared supertiles

            def shared_gu(st, xTs, hsT, spool, pspool):
                """gate/up + silu for one shared-expert supertile."""
                for it in range(ISHT):
                    ps_g3 = pspool.tile([P, STT], f32, tag="ps_g3", bufs=2)
                    ps_u3 = pspool.tile([P, STT], f32, tag="ps_u3", bufs=2)
                    for hc in range(HC):
                        nc.tensor.matmul(
                            out=ps_g3[:], lhsT=sgw[:, hc, it * P:(it + 1) * P],
                            rhs=xTs[:, hc, :],
                            start=(hc == 0), stop=(hc == HC - 1))
                        nc.tensor.matmul(
                            out=ps_u3[:], lhsT=suw[:, hc, it * P:(it + 1) * P],
                            rhs=xTs[:, hc, :],
                            start=(hc == 0), stop=(hc == HC - 1))
                    sil3 = spool.tile([P, STT], f32, tag="sil3")
                    nc.scalar.activation(out=sil3[:], in_=ps_g3[:],
                                         func=Act.Sigmoid)
                    nc.vector.tensor_tensor(out=sil3[:], in0=sil3[:],
                                            in1=ps_g3[:], op=Alu.mult)
                    nc.vector.tensor_tensor(out=hsT[:, it, :], in0=sil3[:],
                                            in1=ps_u3[:], op=Alu.mult)

            # ---------------- P1: XBAR transpose + fp16-split router --------
            with tc.tile_pool(name="p1", bufs=2) as p1, \
                 tc.tile_pool(name="p1ps", bufs=1, space="PSUM") as p1ps:
                for tci in range(TC):
                    t0 = tci * STT
                    xTh = p1.tile([P, HC, STT], f16, tag="xTh")
                    nc.sync.dma_start(
                        out=xTh[:],
                        in_=xht_in[:, t0:t0 + STT].rearrange(
                            "(hc hp) t -> hp hc t", hp=P))
                    xTl = p1.tile([P, HC, STT], f16, tag="xTl")
                    nc.scalar.dma_start(
                        out=xTl[:],
                        in_=xlt_in[:, t0:t0 + STT].rearrange(
                            "(hc hp) t -> hp hc t", hp=P))
                    zps = p1ps.tile([E, STT], f32, tag="zps", bufs=2)
                    pairs = [(rwh_sb, xTh), (rwh_sb, xTl), (rwl_sb, xTh)]
                    n_mm = len(pairs) * HC
                    k = 0
                    for (wsb, xsb) in pairs:
                        for hc in range(HC):
                            nc.tensor.matmul(out=zps[:], lhsT=wsb[:, hc, :],
                                             rhs=xsb[:, hc, :],
                                             start=(k == 0), stop=(k == n_mm - 1))
                            k += 1
                    ztmp = p1.tile([E, STT], f32, tag="ztmp")
                    nc.vector.tensor_copy(out=ztmp[:], in_=zps[:])
                    for q in range(ST):
                        ztr = p1ps.tile([P, E], f32, tag="ztr", bufs=2)
                        nc.tensor.transpose(ztr[:], ztmp[:, q * P:(q + 1) * P],
                                            ident[:E, :E])
                        nc.vector.tensor_copy(out=z_all[:, tci * ST + q, :],
                                              in_=ztr[:])
                    # sum(x^2) stream for the RMS scale (scalar engine)
                    for q in range(ST):
                        j = tci * ST + q
                        xr = p1.tile([P, H], f16, tag="xr", bufs=3)
                        nc.sync.dma_start(out=xr[:],
                                          in_=xh_in[j * P:(j + 1) * P, :])
                        sq = p1.tile([P, H], f32, tag="sq")
                        nc.scalar.activation(out=sq[:], in_=xr[:],
                                             func=Act.Square,
                                             accum_out=ss_all[:, j:j + 1])

            # ---------------- R2: top-2 + weights ---------------------------
            with tc.tile_pool(name="r2", bufs=1) as r2, \
                 tc.tile_pool(name="r2ps", bufs=1, space="PSUM") as r2ps:
                srt = r2.tile([P, J], f32)
                nc.scalar.activation(out=srt[:], in_=ss_all[:], func=Act.Sqrt,
                                     scale=1.0 / H, bias=eps_sb[:])
                nc.vector.reciprocal(out=rms_all[:], in_=srt[:])

                m1 = r2.tile([P, J], f32)
                m2 = r2.tile([P, J], f32)
                idx1 = r2.tile([P, J], f32)
                idx2 = r2.tile([P, J], f32)
                eq = r2.tile([P, J, E], f32)
                tmpje = r2.tile([P, J, E], f32)
                tmp = r2.tile([P, J], f32)
                rw1 = r2.tile([P, J], f32)
                rw2 = r2.tile([P, J], f32)

                nc.vector.tensor_reduce(out=m1[:], in_=z_all[:],
                                        axis=mybir.AxisListType.X, op=Alu.max)
                m1b = m1[:].rearrange("p j -> p j ()").to_broadcast([P, J, E])
                nc.vector.tensor_tensor(out=eq[:], in0=z_all[:], in1=m1b,
                                        op=Alu.is_ge)
                # idx1 = min over e of (eq ? iota : 9)
                nc.vector.scalar_tensor_tensor(out=tmpje[:], in0=iotaf_sb[:],
                                               scalar=-9.0, in1=eq[:],
                                               op0=Alu.add, op1=Alu.mult)
                nc.vector.tensor_scalar_add(tmpje[:], tmpje[:], 9.0)
                nc.vector.tensor_reduce(out=idx1[:], in_=tmpje[:],
                                        axis=mybir.AxisListType.X, op=Alu.min)
                # mask out the top-1 positions, then find second max
                nc.vector.scalar_tensor_tensor(out=eq[:], in0=eq[:],
                                               scalar=-1e30, in1=z_all[:],
                                               op0=Alu.mult, op1=Alu.add)
                nc.vector.tensor_reduce(out=m2[:], in_=eq[:],
                                        axis=mybir.AxisListType.X, op=Alu.max)
                m2b = m2[:].rearrange("p j -> p j ()").to_broadcast([P, J, E])
                nc.vector.tensor_tensor(out=eq[:], in0=eq[:], in1=m2b,
                                        op=Alu.is_ge)
                nc.vector.scalar_tensor_tensor(out=tmpje[:], in0=iotaf_sb[:],
                                               scalar=-9.0, in1=eq[:],
                                               op0=Alu.add, op1=Alu.mult)
                nc.vector.tensor_scalar_add(tmpje[:], tmpje[:], 9.0)
                nc.vector.tensor_reduce(out=idx2[:], in_=tmpje[:],
                                        axis=mybir.AxisListType.X, op=Alu.min)

                # rw1 = sigmoid((m1-m2)*rms), rw2 = 1-rw1
                nc.vector.tensor_sub(tmp[:], m1[:], m2[:])
                nc.vector.tensor_mul(tmp[:], tmp[:], rms_all[:])
                nc.scalar.activation(out=rw1[:], in_=tmp[:], func=Act.Sigmoid)
                nc.vector.tensor_scalar(rw2[:], rw1[:], -1.0, 1.0,
                                        op0=Alu.mult, op1=Alu.add)

                se1 = r2.tile([P, J], f32)
                se2 = r2.tile([P, J], f32)
                sel = r2.tile([P, J], f32)
                nc.vector.tensor_tensor(out=se1[:], in0=idx1[:], in1=eidf_sb[:],
                                        op=Alu.is_equal)
                nc.vector.tensor_tensor(out=se2[:], in0=idx2[:], in1=eidf_sb[:],
                                        op=Alu.is_equal)
                nc.vector.tensor_add(sel[:], se1[:], se2[:])
                nc.vector.tensor_mul(se1[:], se1[:], rw1[:])
                nc.vector.tensor_mul(se2[:], se2[:], rw2[:])
                nc.vector.tensor_add(wv_pm[:], se1[:], se2[:])

                # ------------- R3: capacity dispatch ------------------------
                selT_ps = r2ps.tile([J, P], f32)
                nc.tensor.transpose(selT_ps[:], sel[:], ident[:])
                selT = r2.tile([J, P], f32)
                nc.vector.tensor_copy(out=selT[:], in_=selT_ps[:])
                zerosT = r2.tile([J, P], f32)
                nc.vector.memset(zerosT[:], 0.0)
                rowsum = r2.tile([J, 1], f32)
                nc.vector.tensor_reduce(out=rowsum[:], in_=selT[:],
                                        axis=mybir.AxisListType.X, op=Alu.add)
                offs_ps = r2ps.tile([J, 1], f32)
                nc.tensor.matmul(out=offs_ps[:], lhsT=utri_sb[:], rhs=rowsum[:],
                                 start=True, stop=True)
                scanT = r2.tile([J, P], f32)
                nc.vector.tensor_tensor_scan(out=scanT[:], data0=selT[:],
                                             data1=zerosT[:],
                                             initial=offs_ps[:],
                                             op0=Alu.add, op1=Alu.add)
                nc.vector.tensor_scalar_add(scanT[:], scanT[:], -1.0)
                slot_ps = r2ps.tile([P, J], f32)
                nc.tensor.transpose(slot_ps[:], scanT[:], ident[:J, :J])
                slot_pm = r2.tile([P, J], f32)
                nc.vector.tensor_copy(out=slot_pm[:], in_=slot_ps[:])

                # wv *= (slot < CAP)
                gate = r2.tile([P, J], f32)
                nc.vector.tensor_scalar(gate[:], slot_pm[:], float(CAP), None,
                                        op0=Alu.is_lt)
                nc.vector.tensor_mul(wv_pm[:], wv_pm[:], gate[:])
                # gather slot: clamp to [0, CAP-1]
                sg_f = r2.tile([P, J], f32)
                nc.vector.tensor_scalar(sg_f[:], slot_pm[:], 0.0, float(CAP - 1),
                                        op0=Alu.max, op1=Alu.min)
                nc.vector.tensor_copy(out=slotg_i32[:], in_=sg_f[:])
                # dispatch build: stream-compact (sel & slot<CAP ? tokid : -1)
                # in token order via gpsimd sparse_gather. The capacity cap
                # keeps the found count <= CAP so the ucode cannot overrun
                # its [16, CAP/16] output.
                selcap = r2.tile([P, J], f32)
                nc.vector.tensor_mul(selcap[:], sel[:], gate[:])
                val_pm = r2.tile([P, J], f32)
                nc.vector.scalar_tensor_tensor(out=val_pm[:], in0=tokidf_sb[:],
                                               scalar=1.0, in1=selcap[:],
                                               op0=Alu.add, op1=Alu.mult)
                nc.vector.tensor_scalar_add(val_pm[:], val_pm[:], -1.0)
                nc.sync.dma_start(
                    out=vlin_dram[:].rearrange("(j p) -> p j", p=P),
                    in_=val_pm[:])
                v16 = r2.tile([16, T // 16], f32)
                nc.sync.dma_start(out=v16[:],
                                  in_=vlin_dram[:].rearrange("(f p) -> p f", p=16))
                d16 = r2.tile([16, CAP // 16], f32)
                nfound = r2.tile([1, 1], dt.uint32)
                nc.gpsimd.sparse_gather(out=d16[:], in_=v16[:],
                                        num_found=nfound[:])
                nc.sync.dma_start(
                    out=dlin_dram[:].rearrange("(f p) -> p f", p=16),
                    in_=d16[:])
                dispf = r2.tile([P, CT], f32)
                nc.sync.dma_start(
                    out=dispf[:],
                    in_=dlin_dram[:].rearrange("(ct p) -> p ct", p=P))
                nfoundf = r2.tile([1, 1], f32)
                nc.vector.tensor_copy(out=nfoundf[:], in_=nfound[:])
                cntb = r2.tile([P, 1], f32)
                nc.gpsimd.partition_broadcast(cntb[:], nfoundf[:])
                cgate = r2.tile([P, CT], dt.uint8)
                nc.vector.tensor_scalar(cgate[:], slotiota_sb[:], cntb[:],
                                        None, op0=Alu.is_lt)
                zct = r2.tile([P, CT], f32)
                nc.vector.memset(zct[:], 0.0)
                dsafe = r2.tile([P, CT], f32)
                nc.vector.select(out=dsafe[:], mask=cgate[:], on_true=dispf[:],
                                 on_false=zct[:])
                nc.vector.tensor_copy(out=disp_sb[:], in_=dsafe[:])

            # ------------- prefill shared supertiles 0,1 (hides R3) ---------
            with tc.tile_pool(name="pf", bufs=2) as pf, \
                 tc.tile_pool(name="pfps", bufs=1, space="PSUM") as pfps:
                for st in range(NPF):
                    xTs = pf.tile([P, HC, STT], f16, tag="xTs")
                    nc.sync.dma_start(
                        out=xTs[:],
                        in_=xht_in[:, st * STT:(st + 1) * STT].rearrange(
                            "(hc hp) t -> hp hc t", hp=P))
                    shared_gu(st, xTs, hsT01[st], pf, pfps)

            # ---------------- P2: routed expert FFN -------------------------
            with tc.tile_pool(name="p2", bufs=3) as p2, \
                 tc.tile_pool(name="p2big", bufs=1) as p2big:
                xgT = p2big.tile([P, HC, CAP], f16)
                hT = p2big.tile([P, IRT, CAP], f16)
                with tc.tile_pool(name="p2psA", bufs=1, space="PSUM") as psa:
                    for ct in range(CT):
                        xg_sb = p2.tile([P, H], f16, tag="xg_sb")
                        nc.gpsimd.indirect_dma_start(
                            out=xg_sb[:], out_offset=None,
                            in_=xh_in[:, :],
                            in_offset=bass.IndirectOffsetOnAxis(
                                ap=disp_sb[:, ct:ct + 1], axis=0))
                        nc.sync.dma_start_transpose(
                            out=xgT[:, :, ct * P:(ct + 1) * P], in_=xg_sb[:])

                    cchunks = []
                    c0 = 0
                    while c0 < CAP:
                        cw = min(512, CAP - c0)
                        cchunks.append((c0, cw))
                        c0 += cw
                    for irt in range(IRT):
                        rgw = p2.tile([P, HC, P], f16, tag="rgw")
                        nc.sync.dma_start(
                            out=rgw[:],
                            in_=rg_in[:, irt * P:(irt + 1) * P].rearrange(
                                "(hc hp) i -> hp hc i", hp=P))
                        ruw = p2.tile([P, HC, P], f16, tag="ruw")
                        nc.sync.dma_start(
                            out=ruw[:],
                            in_=ru_in[:, irt * P:(irt + 1) * P].rearrange(
                                "(hc hp) i -> hp hc i", hp=P))
                        for (c0, cw) in cchunks:
                            ps_g = psa.tile([P, 512], f32, tag="ps_g", bufs=2)
                            ps_u = psa.tile([P, 512], f32, tag="ps_u", bufs=2)
                            for hc in range(HC):
                                nc.tensor.matmul(
                                    out=ps_g[:, :cw], lhsT=rgw[:, hc, :],
                                    rhs=xgT[:, hc, c0:c0 + cw],
                                    start=(hc == 0), stop=(hc == HC - 1))
                                nc.tensor.matmul(
                                    out=ps_u[:, :cw], lhsT=ruw[:, hc, :],
                                    rhs=xgT[:, hc, c0:c0 + cw],
                                    start=(hc == 0), stop=(hc == HC - 1))
                            sil = p2.tile([P, 512], f32, tag="sil")
                            nc.scalar.activation(out=sil[:, :cw],
                                                 in_=ps_g[:, :cw],
                                                 func=Act.Sigmoid)
                            nc.vector.tensor_tensor(
                                out=sil[:, :cw], in0=sil[:, :cw],
                                in1=ps_g[:, :cw], op=Alu.mult)
                            nc.vector.tensor_tensor(
                                out=hT[:, irt, c0:c0 + cw], in0=sil[:, :cw],
                                in1=ps_u[:, :cw], op=Alu.mult)

                # down: eout[c, h] = sum_ir h[c, ir] * rd[ir, h]
                with tc.tile_pool(name="p2psD", bufs=1, space="PSUM") as psd:
                    ct0 = 0
                    while ct0 < CT:
                        blk = min(4, CT - ct0)
                        ps_d = [[psd.tile([P, 512], f32, tag=f"d{i}{hn}",
                                          name=f"d{i}{hn}")
                                 for hn in range(NHT)] for i in range(blk)]
                        for ic in range(IRT):
                            rdw = p2.tile([P, H], f16, tag="rdw")
                            nc.sync.dma_start(
                                out=rdw[:], in_=rd_in[ic * P:(ic + 1) * P, :])
                            for i in range(blk):
                                for hn in range(NHT):
                                    nc.tensor.matmul(
                                        out=ps_d[i][hn][:],
                                        lhsT=hT[:, ic,
                                                (ct0 + i) * P:(ct0 + i + 1) * P],
                                        rhs=rdw[:, hn * 512:(hn + 1) * 512],
                                        start=(ic == 0), stop=(ic == IRT - 1))
                        for i in range(blk):
                            eo = p2.tile([P, H], f16, tag="eo")
                            for hn in range(NHT):
                                nc.vector.tensor_copy(
                                    out=eo[:, hn * 512:(hn + 1) * 512],
                                    in_=ps_d[i][hn][:])
                            nc.sync.dma_start(
                                out=eout_dram[(ct0 + i) * P:(ct0 + i + 1) * P, :],
                                in_=eo[:])
                        ct0 += blk

            # ---------------- P3: shared expert + combine + RS --------------
            with tc.tile_pool(name="p3w", bufs=1) as p3w, \
                 tc.tile_pool(name="p3", bufs=2) as p3, \
                 tc.tile_pool(name="p3ps", bufs=1, space="PSUM") as p3ps:
                sdw = p3w.tile([P, ISHT, H], f16)
                nc.sync.dma_start(
                    out=sdw[:],
                    in_=sd_in[:].rearrange("(it ip) h -> ip it h", ip=P))

                for st in range(NST):
                    if st < NPF:
                        hsT = hsT01[st]
                    else:
                        xTs = p3.tile([P, HC, STT], f16, tag="xTs")
                        nc.sync.dma_start(
                            out=xTs[:],
                            in_=xht_in[:, st * STT:(st + 1) * STT].rearrange(
                                "(hc hp) t -> hp hc t", hp=P))
                        hsT = p3.tile([P, ISHT, STT], f16, tag="hsT")
                        shared_gu(st, xTs, hsT, p3, p3ps)
                    for ts in range(ST):
                        j = st * ST + ts
                        gath = p3.tile([P, H], f16, tag="gath")
                        nc.gpsimd.indirect_dma_start(
                            out=gath[:], out_offset=None,
                            in_=eout_dram[:, :],
                            in_offset=bass.IndirectOffsetOnAxis(
                                ap=slotg_i32[:, j:j + 1], axis=0))
                        outt = p3.tile([P, H], f16, tag="outt")
                        for hn in range(NHT):
                            ps_d3 = p3ps.tile([P, 512], f32, tag="ps_d3",
                                              bufs=2)
                            for it in range(ISHT):
                                nc.tensor.matmul(
                                    out=ps_d3[:],
                                    lhsT=hsT[:, it, ts * P:(ts + 1) * P],
                                    rhs=sdw[:, it, hn * 512:(hn + 1) * 512],
                                    start=(it == 0), stop=(it == ISHT - 1))
                            nc.vector.scalar_tensor_tensor(
                                out=outt[:, hn * 512:(hn + 1) * 512],
                                in0=gath[:, hn * 512:(hn + 1) * 512],
                                scalar=wv_pm[:, j:j + 1],
                                in1=ps_d3[:],
                                op0=Alu.mult, op1=Alu.add)
                        k = j // JCH
                        r = j % JCH
                        nc.sync.dma_start(
                            out=chunk_dram[k][r * P:(r + 1) * P, :],
                            in_=outt[:])
                    if (st + 1) % (NST // NCH) == 0:
                        k = (st + 1) // (NST // NCH) - 1
                        nc.gpsimd.collective_compute(
                            "ReduceScatter", Alu.add,
                            replica_groups=replica_groups,
                            ins=[chunk_dram[k][:]],
                            outs=[rsout_dram[k][:]])
                        # fp16 -> f32 cast on the way out (SWDGE)
                        nc.gpsimd.dma_start(out=o_out[k], in_=rsout_dram[k][:])

    nc.finalize()
    return nc


_NC_CACHE = {}


def _get_nc(key="full"):
    if key not in _NC_CACHE:
        _NC_CACHE[key] = build(FULL_CFG)
    return _NC_CACHE[key]


def make_in_maps(inputs, cfg=FULL_CFG):
    T, H, E, IR, IS, CAP, NCORES = (
        cfg[k] for k in ("T", "H", "E", "IR", "IS", "CAP", "NC"))
    J = T // P
    ISH = IS // NCORES
    f16 = np.float16
    x = np.ascontiguousarray(np.asarray(inputs["x"], np.float32).reshape(T, H))
    xh = x.astype(f16)
    xl = (x - xh.astype(np.float32)).astype(f16)
    xht = np.ascontiguousarray(xh.T)
    xlt = np.ascontiguousarray(xl.T)
    rw = np.asarray(inputs["router_w"], np.float32)
    rwt = np.ascontiguousarray(rw.T)
    rwh = rwt.astype(f16)
    rwl = (rwt - rwh.astype(np.float32)).astype(f16)
    rg = np.asarray(inputs["rg"], np.float32)
    ru = np.asarray(inputs["ru"], np.float32)
    rd = np.asarray(inputs["rd"], np.float32)
    sg = np.asarray(inputs["sg"], np.float32)
    su = np.asarray(inputs["su"], np.float32)
    sd = np.asarray(inputs["sd"], np.float32)

    tokidf = np.ascontiguousarray(
        np.arange(T, dtype=np.float32).reshape(J, P).T)      # [p, j] = 128j+p
    slotiota = np.ascontiguousarray(
        np.arange(CAP, dtype=np.float32).reshape(CAP // P, P).T)  # [p, ct]
    utri = np.triu(np.ones((J, J), np.float32), k=1)
    iotaf = np.tile(np.arange(E, dtype=np.float32), (P, J))  # [P, J*E]

    in_maps = []
    for i in range(NCORES):
        in_maps.append(dict(
            xh=xh, xht=xht, xlt=xlt, rwh=rwh, rwl=rwl,
            rg=np.ascontiguousarray(rg[i]).astype(f16),
            ru=np.ascontiguousarray(ru[i]).astype(f16),
            rd=np.ascontiguousarray(rd[i]).astype(f16),
            sg=np.ascontiguousarray(sg[:, i * ISH:(i + 1) * ISH]).astype(f16),
            su=np.ascontiguousarray(su[:, i * ISH:(i + 1) * ISH]).astype(f16),
            sd=np.ascontiguousarray(sd[i * ISH:(i + 1) * ISH, :]).astype(f16),
            eidf=np.full((P, J), float(i), np.float32),
            iotaf=iotaf, tokidf=tokidf, slotiota=slotiota, utri=utri,
        ))
    return in_maps


def assemble_output(results, cfg=FULL_CFG):
    T, H, NCORES = cfg["T"], cfg["H"], cfg["NC"]
    NCH = cfg.get("NCH", 16)
    rows_per = T // NCH // NCORES
    out = np.empty((T, H), np.float32)
    for i in range(NCORES):
        o = results[i]["o"]
        for k in range(NCH):
            base = (T // NCH) * k + rows_per * i
            out[base:base + rows_per] = o[k]
    return out


def kernel(**inputs):
    nc = _get_nc()
    in_maps = make_in_maps(inputs)
    core_ids = list(range(FULL_CFG["NC"]))
    last_err = None
    for _attempt in range(2):
        try:
            res = run_bass_kernel_spmd(nc, in_maps, core_ids, trace=False)
            break
        except Exception as e:  # transient device wedges: retry once
            last_err = e
    else:
        raise last_err
    out = assemble_output(res.results)
    B, S, H = 4, 2048, 1024
    return out.reshape(B, S, H)
